# revision 13
# baseline (speedup 1.0000x reference)
"""Trainium2 Bass kernel for nn_Block_18064632447630 (sparse_attention).

Sharding: 8 cores = batch(4) x seq-half(2). Each core independently computes
2048 rows of one batch: sparse self-attention (keys gathered host-side at the
128 selected positions), cross-attention over the class vector, and the FFN.
Activations flow feature-major [feat, rows] on-device so every matmul uses
native-layout weights as the stationary operand and no on-device transposes
are needed. Partition-dim reductions (softmax denominators, LN stats) are done
with ones/indicator matmuls on the TensorEngine; partition broadcasts with
K=1 matmuls. Matmul operands are bf16 (f32 PSUM accumulation).
"""

import sys

sys.path.insert(0, "/opt/trn_rl_repo")

import numpy as np
import ml_dtypes

import concourse.bass as bass
import concourse.mybir as mybir
import concourse.tile as tile
from concourse import bacc
from concourse.bass_utils import run_bass_kernel_spmd

BF16 = ml_dtypes.bfloat16
F32, BF = mybir.dt.float32, mybir.dt.bfloat16
AF = mybir.ActivationFunctionType
ALU = mybir.AluOpType

B, S, D, H, DH, G, C, FF = 4, 4096, 768, 12, 64, 64, 256, 3072
S2 = S // 2          # rows per core
RC = 512             # row-chunk (matmul free dim)
NRC = S2 // RC       # 4 row chunks
C6 = D // 128        # 6 feature chunks
F24 = FF // 128      # 24 ff chunks
J = 2 * G            # 128 selected keys
CL2 = C // 128       # 2 class chunks
SCALE = 0.125        # 1/sqrt(DH)

_NC_CACHE = {}


def _build_nc():
    nc = bacc.Bacc(None, target_bir_lowering=False, debug=False)
    P = {}

    def param(name, shape, dt, out=False):
        P[name] = nc.declare_dram_parameter(name, shape, dt, isOutput=out)

    param("xTb", [D, S2], BF)
    param("kvTb", [D, J], BF)
    param("selv", [128, 1], F32)
    param("cls_row_b", [1, C], BF)
    for w in ("Wq", "Wk", "Wv", "Wo", "Wqc", "Woc"):
        param(w, [D, D], BF)
    param("W1", [D, FF], BF)
    param("W2", [FF, D], BF)
    param("Wkc_b", [1, D], BF)
    param("Wvc_b", [1, D], BF)
    for b in ("bk_col", "bkc_col", "bo_col", "boc_col", "bf2_col"):
        param(b, [128, C6], F32)
    param("bf1_col", [128, F24], F32)
    param("bq_colb", [128, C6], BF)
    param("bqc_colb", [128, C6], BF)
    for r in ("bv_row", "bvc_row", "g1_row", "b1_row", "g2_row", "b2_row",
              "g3_row", "b3_row"):
        param(r, [1, D], BF)
    param("IndT", [H, C6 * 128], BF)
    param("out", [D, S2], F32, out=True)

    with tile.TileContext(nc) as tc:
        with nc.allow_low_precision(reason="bf16 activations; rel-err gate 2e-2"):
            _body(nc, tc, P)
    nc.compile()
    return nc


def _body(nc, tc, P):
    from contextlib import ExitStack
    ctx = ExitStack()
    cpool = ctx.enter_context(tc.tile_pool(name="consts", bufs=1))
    wpool = ctx.enter_context(tc.tile_pool(name="weights", bufs=2))
    apool = ctx.enter_context(tc.tile_pool(name="acts", bufs=2))
    ps = ctx.enter_context(tc.tile_pool(name="psum", bufs=1, space="PSUM"))

    def acc_ps(i):
        return ps.tile([128, RC], F32, tag=f"acc{i}", name=f"acc{i}", bufs=1)

    def aux_ps(shape, name):
        return ps.tile(shape, F32, tag="aux", name=name, bufs=2)

    # ---------- constants / small inputs ----------
    def load_const(name, shape, dt, src):
        t = cpool.tile(shape, dt, name=name)
        nc.sync.dma_start(t, src)
        return t

    selv = load_const("selv", [128, 1], F32, P["selv"][:])
    cls_row = load_const("cls_row", [1, C], BF, P["cls_row_b"][:])
    bk_col = load_const("bk_col", [128, C6], F32, P["bk_col"][:])
    bkc_col = load_const("bkc_col", [128, C6], F32, P["bkc_col"][:])
    bo_col = load_const("bo_col", [128, C6], F32, P["bo_col"][:])
    boc_col = load_const("boc_col", [128, C6], F32, P["boc_col"][:])
    bf2_col = load_const("bf2_col", [128, C6], F32, P["bf2_col"][:])
    bf1_col = load_const("bf1_col", [128, F24], F32, P["bf1_col"][:])
    bq_colb = load_const("bq_colb", [128, C6], BF, P["bq_colb"][:])
    bqc_colb = load_const("bqc_colb", [128, C6], BF, P["bqc_colb"][:])
    rows = {}
    for r in ("bv_row", "bvc_row", "g1_row", "b1_row", "g2_row", "b2_row",
              "g3_row", "b3_row"):
        rows[r] = load_const(r, [1, D], BF, P[r][:])
    wkc_row = load_const("wkc_row", [1, D], BF, P["Wkc_b"][:])
    wvc_row = load_const("wvc_row", [1, D], BF, P["Wvc_b"][:])
    kvTb = load_const("kvTb", [128, C6, J], BF,
                      P["kvTb"][:].rearrange("(c p) j -> p c j", p=128))

    ones1 = cpool.tile([1, 128], BF, name="ones1")
    nc.vector.memset(ones1, 1.0)
    ones_col = cpool.tile([128, 1], BF, name="ones_col")
    nc.vector.memset(ones_col, 1.0)
    ones_row = cpool.tile([1, RC], BF, name="ones_row")
    nc.vector.memset(ones_row, 1.0)
    eps_t = cpool.tile([1, 1], F32, name="eps_t")
    nc.vector.memset(eps_t, 1e-5)

    E_all = cpool.tile([128, H, H], BF, name="E_all")
    nc.vector.memset(E_all, 0.0)
    for h in range(H):
        nc.vector.memset(E_all[:, h, h : h + 1], 1.0)
    IndT = cpool.tile([H, C6, 128], BF, name="IndT")
    nc.sync.dma_start(IndT, P["IndT"][:].rearrange("h (c n) -> h c n", n=128))

    iota_i = cpool.tile([128, RC], mybir.dt.int32, name="iota_i")
    nc.gpsimd.iota(iota_i, pattern=[[1, RC]], base=0, channel_multiplier=0)
    iota_f = cpool.tile([128, RC], F32, name="iota_f")
    nc.vector.tensor_copy(iota_f, iota_i)

    # ---------- K/V projection at the 128 selected positions ----------
    KTb = cpool.tile([128, C6, J], BF, name="KTb")
    Wk_t = wpool.tile([128, C6, D], BF, tag="w768", name="Wk_t")
    nc.sync.dma_start(Wk_t, P["Wk"][:].rearrange("(c p) n -> p c n", p=128))
    for co in range(C6):
        pk = aux_ps([128, J], f"pk{co}")
        for kc in range(C6):
            nc.tensor.matmul(pk, Wk_t[:, kc, co * 128 : co * 128 + 128],
                             kvTb[:, kc, :], start=(kc == 0), stop=(kc == C6 - 1))
        nc.vector.tensor_scalar(KTb[:, co, :], pk, bk_col[:, co : co + 1], None,
                                ALU.add)

    Vb = cpool.tile([128, D], BF, name="Vb")
    Wv_t = wpool.tile([128, C6, D], BF, tag="w768", name="Wv_t")
    nc.sync.dma_start(Wv_t, P["Wv"][:].rearrange("(c p) n -> p c n", p=128))
    for ns, nw in ((0, 512), (512, 256)):
        pv = aux_ps([128, 512], f"pv{ns}")
        for kc in range(C6):
            nc.tensor.matmul(pv[:, :nw], kvTb[:, kc, :],
                             Wv_t[:, kc, ns : ns + nw], start=(kc == 0), stop=False)
        nc.tensor.matmul(pv[:, :nw], ones1, rows["bv_row"][:, ns : ns + nw],
                         start=False, stop=True)
        nc.scalar.activation(Vb[:, ns : ns + nw], pv[:, :nw], AF.Copy)

    # kc^T[d, cl] = Wkc[d]*cls[cl] + bkc[d]  (rank-1, K=1 matmul)
    kcb = cpool.tile([128, C6, C], BF, name="kcb")
    for c in range(C6):
        pkc = aux_ps([128, C], f"pkc{c}")
        nc.tensor.matmul(pkc, wkc_row[:, c * 128 : c * 128 + 128], cls_row,
                         start=True, stop=True)
        nc.vector.tensor_scalar(kcb[:, c, :], pkc, bkc_col[:, c : c + 1], None,
                                ALU.add)
    # vc[cl, d] = cls[cl]*Wvc[d] + bvc[d]
    vcb = cpool.tile([128, CL2, D], BF, name="vcb")
    for clc in range(CL2):
        for ns, nw in ((0, 512), (512, 256)):
            pvc = aux_ps([128, 512], f"pvc{clc}_{ns}")
            nc.tensor.matmul(pvc[:, :nw], cls_row[:, clc * 128 : clc * 128 + 128],
                             wvc_row[:, ns : ns + nw], start=True, stop=False)
            nc.tensor.matmul(pvc[:, :nw], ones1, rows["bvc_row"][:, ns : ns + nw],
                             start=False, stop=True)
            nc.scalar.activation(vcb[:, clc, ns : ns + nw], pvc[:, :nw], AF.Copy)

    # exp-bias folds: qk_bias[j] = SCALE*(bq . k_j); qkc_bias[cl] = SCALE*(bqc . kc_cl)
    qk_bias = cpool.tile([128, 1], F32, name="qk_bias")
    pqb = aux_ps([128, 1], "pqb")
    for c in range(C6):
        nc.tensor.matmul(pqb, KTb[:, c, :], bq_colb[:, c : c + 1],
                         start=(c == 0), stop=(c == C6 - 1))
    nc.vector.tensor_scalar(qk_bias, pqb, SCALE, None, ALU.mult)
    qkc_bias = cpool.tile([128, CL2], F32, name="qkc_bias")
    for clc in range(CL2):
        pqc = aux_ps([128, 1], f"pqcb{clc}")
        for c in range(C6):
            nc.tensor.matmul(pqc, kcb[:, c, clc * 128 : clc * 128 + 128],
                             bqc_colb[:, c : c + 1], start=(c == 0),
                             stop=(c == C6 - 1))
        nc.vector.tensor_scalar(qkc_bias[:, clc : clc + 1], pqc, SCALE, None,
                                ALU.mult)

    xT_d = P["xTb"][:].rearrange("(c p) s -> p c s", p=128)
    out_d = P["out"][:].rearrange("(c p) s -> p c s", p=128)

    # ---------- per row-chunk pipeline ----------
    for rc in range(NRC):
        sl = slice(rc * RC, rc * RC + RC)
        xtb = apool.tile([128, C6, RC], BF, tag="xtb", name=f"xtb{rc}")
        nc.sync.dma_start(xtb, xT_d[:, :, sl])

        # mask[j, s] = (iota >= sel[j] - rc*RC)  as bf16 0/1
        selv_sh = apool.tile([128, 1], F32, tag="selv_sh", name=f"ssh{rc}")
        nc.vector.tensor_scalar(selv_sh, selv, float(-rc * RC), None, ALU.add)
        mask = apool.tile([128, RC], BF, tag="mask", name=f"mask{rc}")
        nc.vector.tensor_scalar(mask, iota_f, selv_sh, None, ALU.is_ge)

        # --- Q projection (feature-major): qTb = Wq^T-chunks @ xtb ---
        Wq_t = wpool.tile([128, C6, D], BF, tag="w768", name=f"Wq{rc}")
        nc.sync.dma_start(Wq_t, P["Wq"][:].rearrange("(c p) n -> p c n", p=128))
        qTb = apool.tile([128, C6, RC], BF, tag="qTb", name=f"qTb{rc}", bufs=1)
        for co in range(C6):
            pq = aux_ps([128, RC], f"pq{rc}_{co}")
            for kc in range(C6):
                nc.tensor.matmul(pq, Wq_t[:, kc, co * 128 : co * 128 + 128],
                                 xtb[:, kc, :], start=(kc == 0),
                                 stop=(kc == C6 - 1))
            nc.scalar.activation(qTb[:, co, :], pq, AF.Copy)

        # --- sparse attention ---
        pts = []
        for h in range(H):
            psc = aux_ps([128, RC], f"psc{rc}_{h}")
            nc.tensor.matmul(psc, KTb[64 * (h % 2) : 64 * (h % 2) + 64, h // 2, :],
                             qTb[64 * (h % 2) : 64 * (h % 2) + 64, h // 2, :],
                             start=True, stop=True)
            pt = apool.tile([128, RC], BF, tag="pt", name=f"pt{rc}_{h}", bufs=12)
            nc.scalar.activation(pt, psc, AF.Exp, bias=qk_bias, scale=SCALE)
            nc.vector.tensor_tensor(pt, pt, mask, ALU.mult)
            pts.append(pt)
        pden = aux_ps([H, RC], f"pden{rc}")
        for h in range(H):
            nc.tensor.matmul(pden, E_all[:, h, :], pts[h], start=(h == 0),
                             stop=(h == H - 1))
        recd = apool.tile([H, RC], BF, tag="recd", name=f"recd{rc}", bufs=1)
        nc.vector.reciprocal(recd, pden)
        po = [acc_ps(c) for c in range(C6)]
        for h in range(H):
            c, lo = h // 2, 64 * (h % 2)
            nc.tensor.matmul(po[c][lo : lo + 64, :],
                             Vb[:, c * 128 + lo : c * 128 + lo + 64], pts[h],
                             start=True, stop=True)
        oTb = apool.tile([128, C6, RC], BF, tag="oTb", name=f"oTb{rc}", bufs=1)
        for c in range(C6):
            prb = aux_ps([128, RC], f"prb{rc}_{c}")
            nc.tensor.matmul(prb, IndT[:, c, :], recd, start=True, stop=True)
            rb = apool.tile([128, RC], BF, tag="rb", name=f"rb{rc}_{c}")
            nc.scalar.activation(rb, prb, AF.Copy)
            nc.vector.tensor_tensor(oTb[:, c, :], po[c], rb, ALU.mult)

        # --- Wo projection + residual -> r1b ---
        Wo_t = wpool.tile([128, C6, D], BF, tag="w768", name=f"Wo{rc}")
        nc.sync.dma_start(Wo_t, P["Wo"][:].rearrange("(c p) n -> p c n", p=128))
        r1b = apool.tile([128, C6, RC], BF, tag="res", name=f"r1b{rc}", bufs=2)
        for co in range(C6):
            pw = aux_ps([128, RC], f"pwo{rc}_{co}")
            for kc in range(C6):
                nc.tensor.matmul(pw, Wo_t[:, kc, co * 128 : co * 128 + 128],
                                 oTb[:, kc, :], start=(kc == 0),
                                 stop=(kc == C6 - 1))
            tmp = apool.tile([128, RC], BF, tag="tmp", name=f"t1{rc}_{co}")
            nc.vector.tensor_scalar(tmp, pw, bo_col[:, co : co + 1], None, ALU.add)
            nc.vector.tensor_tensor(r1b[:, co, :], tmp, xtb[:, co, :], ALU.add)

        x1b = _layernorm(nc, tc, apool, aux_ps, r1b, rows["g1_row"],
                         rows["b1_row"], ones_col, ones1, ones_row, eps_t,
                         f"ln1_{rc}", F32)

        # --- cross attention ---
        Wqc_t = wpool.tile([128, C6, D], BF, tag="w768", name=f"Wqc{rc}")
        nc.sync.dma_start(Wqc_t, P["Wqc"][:].rearrange("(c p) n -> p c n", p=128))
        qcb = apool.tile([128, C6, RC], BF, tag="qTb", name=f"qcb{rc}", bufs=1)
        for co in range(C6):
            pq = aux_ps([128, RC], f"pqc{rc}_{co}")
            for kc in range(C6):
                nc.tensor.matmul(pq, Wqc_t[:, kc, co * 128 : co * 128 + 128],
                                 x1b[:, kc, :], start=(kc == 0),
                                 stop=(kc == C6 - 1))
            nc.scalar.activation(qcb[:, co, :], pq, AF.Copy)

        pcden = aux_ps([H, RC], f"pcden{rc}")
        poc = [acc_ps(c) for c in range(C6)]
        for half in range(2):
            pcts = {}
            hs = range(half * 6, half * 6 + 6)
            for h in hs:
                c, lo = h // 2, 64 * (h % 2)
                for clc in range(CL2):
                    psc = aux_ps([128, RC], f"pcs{rc}_{h}_{clc}")
                    nc.tensor.matmul(
                        psc, kcb[lo : lo + 64, c, clc * 128 : clc * 128 + 128],
                        qcb[lo : lo + 64, c, :], start=True, stop=True)
                    pct = apool.tile([128, RC], BF, tag="pt",
                                     name=f"pct{rc}_{h}_{clc}", bufs=12)
                    nc.scalar.activation(pct, psc, AF.Exp,
                                         bias=qkc_bias[:, clc : clc + 1],
                                         scale=SCALE)
                    pcts[(h, clc)] = pct
            for h in hs:
                for clc in range(CL2):
                    nc.tensor.matmul(pcden, E_all[:, h, :], pcts[(h, clc)],
                                     start=(h == 0 and clc == 0),
                                     stop=(h == H - 1 and clc == CL2 - 1))
            for h in hs:
                c, lo = h // 2, 64 * (h % 2)
                for clc in range(CL2):
                    nc.tensor.matmul(poc[c][lo : lo + 64, :],
                                     vcb[:, clc, c * 128 + lo : c * 128 + lo + 64],
                                     pcts[(h, clc)], start=(clc == 0),
                                     stop=(clc == CL2 - 1))
        recdc = apool.tile([H, RC], BF, tag="recd", name=f"recdc{rc}", bufs=1)
        nc.vector.reciprocal(recdc, pcden)
        ocb = apool.tile([128, C6, RC], BF, tag="oTb", name=f"ocb{rc}", bufs=1)
        for c in range(C6):
            prb = aux_ps([128, RC], f"pcrb{rc}_{c}")
            nc.tensor.matmul(prb, IndT[:, c, :], recdc, start=True, stop=True)
            rb = apool.tile([128, RC], BF, tag="rb", name=f"crb{rc}_{c}")
            nc.scalar.activation(rb, prb, AF.Copy)
            nc.vector.tensor_tensor(ocb[:, c, :], poc[c], rb, ALU.mult)

        # --- Woc projection + residual -> r2b ---
        Woc_t = wpool.tile([128, C6, D], BF, tag="w768", name=f"Woc{rc}")
        nc.sync.dma_start(Woc_t, P["Woc"][:].rearrange("(c p) n -> p c n", p=128))
        r2b = apool.tile([128, C6, RC], BF, tag="res", name=f"r2b{rc}", bufs=2)
        for co in range(C6):
            pw = aux_ps([128, RC], f"pwoc{rc}_{co}")
            for kc in range(C6):
                nc.tensor.matmul(pw, Woc_t[:, kc, co * 128 : co * 128 + 128],
                                 ocb[:, kc, :], start=(kc == 0),
                                 stop=(kc == C6 - 1))
            tmp = apool.tile([128, RC], BF, tag="tmp", name=f"t2{rc}_{co}")
            nc.vector.tensor_scalar(tmp, pw, boc_col[:, co : co + 1], None,
                                    ALU.add)
            nc.vector.tensor_tensor(r2b[:, co, :], tmp, x1b[:, co, :], ALU.add)

        x2b = _layernorm(nc, tc, apool, aux_ps, r2b, rows["g2_row"],
                         rows["b2_row"], ones_col, ones1, ones_row, eps_t,
                         f"ln2_{rc}", F32)

        # --- FFN ---
        W1_t = wpool.tile([128, C6, FF], BF, tag="w1", name=f"W1{rc}", bufs=1)
        nc.sync.dma_start(W1_t, P["W1"][:].rearrange("(c p) n -> p c n", p=128))
        py = [acc_ps(c) for c in range(C6)]
        for fc in range(F24):
            ph = aux_ps([128, RC], f"ph{rc}_{fc}")
            for kc in range(C6):
                nc.tensor.matmul(ph, W1_t[:, kc, fc * 128 : fc * 128 + 128],
                                 x2b[:, kc, :], start=(kc == 0),
                                 stop=(kc == C6 - 1))
            hb = apool.tile([128, RC], BF, tag="hb", name=f"hb{rc}_{fc}", bufs=3)
            nc.scalar.activation(hb, ph, AF.Relu, bias=bf1_col[:, fc : fc + 1])
            W2f = wpool.tile([128, D], BF, tag="w2f", name=f"W2{rc}_{fc}", bufs=3)
            nc.sync.dma_start(
                W2f, P["W2"][:].rearrange("(f p) n -> p f n", p=128)[:, fc, :])
            for co in range(C6):
                nc.tensor.matmul(py[co], W2f[:, co * 128 : co * 128 + 128], hb,
                                 start=(fc == 0), stop=(fc == F24 - 1))
        r3b = apool.tile([128, C6, RC], BF, tag="res3", name=f"r3b{rc}", bufs=1)
        for co in range(C6):
            tmp = apool.tile([128, RC], BF, tag="tmp", name=f"t3{rc}_{co}")
            nc.vector.tensor_scalar(tmp, py[co], bf2_col[:, co : co + 1], None,
                                    ALU.add)
            nc.vector.tensor_tensor(r3b[:, co, :], tmp, x2b[:, co, :], ALU.add)

        x3 = _layernorm(nc, tc, apool, aux_ps, r3b, rows["g3_row"],
                        rows["b3_row"], ones_col, ones1, ones_row, eps_t,
                        f"ln3_{rc}", F32, out_f32=True)
        for c in range(C6):
            nc.sync.dma_start(out_d[:, c, sl], x3[:, c, :])

    ctx.close()


def _layernorm(nc, tc, apool, aux_ps, rb, g_row, b_row, ones_col, ones1,
               ones_row, eps_t, nm, F32dt, out_f32=False):
    """Feature-major LN over the partition(x6 chunks) axis of rb [128,6,RC]."""
    pstat_s = aux_ps([1, RC], f"psts_{nm}")
    for c in range(C6):
        nc.tensor.matmul(pstat_s, ones_col, rb[:, c, :], start=(c == 0),
                         stop=(c == C6 - 1))
    pstat_q = aux_ps([1, RC], f"pstq_{nm}")
    for c in range(C6):
        sq = apool.tile([128, RC], BF, tag="sq", name=f"sq_{nm}_{c}", bufs=2)
        nc.scalar.activation(sq, rb[:, c, :], AF.Square)
        nc.tensor.matmul(pstat_q, ones_col, sq, start=(c == 0),
                         stop=(c == C6 - 1))
    negm = apool.tile([1, RC], F32dt, tag="negm", name=f"negm_{nm}", bufs=1)
    nc.vector.tensor_scalar(negm, pstat_s, -1.0 / D, None, ALU.mult)
    ex2 = apool.tile([1, RC], F32dt, tag="ex2", name=f"ex2_{nm}", bufs=1)
    nc.vector.tensor_scalar(ex2, pstat_q, 1.0 / D, None, ALU.mult)
    msq = apool.tile([1, RC], F32dt, tag="msq", name=f"msq_{nm}", bufs=1)
    nc.vector.tensor_tensor(msq, negm, negm, ALU.mult)
    var = apool.tile([1, RC], F32dt, tag="var", name=f"var_{nm}", bufs=1)
    nc.vector.tensor_tensor(var, ex2, msq, ALU.subtract)
    std = apool.tile([1, RC], F32dt, tag="std", name=f"std_{nm}", bufs=1)
    nc.scalar.activation(std, var, AF.Sqrt, bias=eps_t)
    rstd = apool.tile([1, RC], F32dt, tag="rstd", name=f"rstd_{nm}", bufs=1)
    nc.vector.reciprocal(rstd, std)
    a_b = apool.tile([1, RC], BF, tag="a_b", name=f"ab_{nm}", bufs=1)
    nc.vector.tensor_copy(a_b, rstd)
    bp_b = apool.tile([1, RC], BF, tag="bp_b", name=f"bp_{nm}", bufs=1)
    nc.vector.tensor_tensor(bp_b, negm, rstd, ALU.mult)
    odt = F32dt if out_f32 else BF
    xout = apool.tile([128, C6, RC], odt, tag="lnout" + ("f" if out_f32 else ""),
                      name=f"xo_{nm}", bufs=(1 if out_f32 else 2))
    for c in range(C6):
        p1 = aux_ps([128, RC], f"p1_{nm}_{c}")
        nc.tensor.matmul(p1, g_row[:, c * 128 : c * 128 + 128], a_b,
                         start=True, stop=True)
        p2 = aux_ps([128, RC], f"p2_{nm}_{c}")
        nc.tensor.matmul(p2, g_row[:, c * 128 : c * 128 + 128], bp_b,
                         start=True, stop=False)
        nc.tensor.matmul(p2, b_row[:, c * 128 : c * 128 + 128], ones_row,
                         start=False, stop=True)
        t = apool.tile([128, RC], BF, tag="lntmp", name=f"lt_{nm}_{c}", bufs=2)
        nc.vector.tensor_tensor(t, rb[:, c, :], p1, ALU.mult)
        nc.vector.tensor_tensor(xout[:, c, :], t, p2, ALU.add)
    return xout


# ---------------- host side ----------------

def _prep_core_inputs(b, half, cur_input, prevLayerOutput, classVector, rand_idx,
                      weights_b):
    s0 = half * S2
    sel = np.concatenate([np.arange(G), np.asarray(rand_idx[b]).astype(np.int64)])
    kv = np.asarray(prevLayerOutput[b])[sel]            # [128, 768]
    m = {
        "xTb": np.ascontiguousarray(np.asarray(cur_input[b])[s0 : s0 + S2].T)
        .astype(BF16),
        "kvTb": np.ascontiguousarray(kv.T).astype(BF16),
        "selv": (sel.astype(np.float32) - s0).reshape(128, 1),
        "cls_row_b": np.asarray(classVector[b]).reshape(1, C).astype(BF16),
    }
    m.update(weights_b)
    return m


def kernel(**inputs):
    if "nc" not in _NC_CACHE:
        _NC_CACHE["nc"] = _build_nc()
    nc = _NC_CACHE["nc"]

    f32 = lambda x: np.asarray(x, dtype=np.float32)
    col = lambda v, c: np.ascontiguousarray(
        f32(v).reshape(c, 128).T).astype(np.float32)
    colb = lambda v, c: col(v, c).astype(BF16)
    row = lambda v: f32(v).reshape(1, -1).astype(BF16)

    indt = np.zeros((H, C6, 128), np.float32)
    for c in range(C6):
        indt[2 * c, c, 0:64] = 1.0
        indt[2 * c + 1, c, 64:128] = 1.0
    wb = {
        "IndT": indt.reshape(H, C6 * 128).astype(BF16),
        "Wq": f32(inputs["Wq"]).astype(BF16),
        "Wk": f32(inputs["Wk"]).astype(BF16),
        "Wv": f32(inputs["Wv"]).astype(BF16),
        "Wo": f32(inputs["Wo"]).astype(BF16),
        "Wqc": f32(inputs["Wqc"]).astype(BF16),
        "Woc": f32(inputs["Woc"]).astype(BF16),
        "W1": f32(inputs["W1"]).astype(BF16),
        "W2": f32(inputs["W2"]).astype(BF16),
        "Wkc_b": row(inputs["Wkc"]),
        "Wvc_b": row(inputs["Wvc"]),
        "bk_col": col(inputs["bk"], C6),
        "bkc_col": col(inputs["bkc"], C6),
        "bo_col": col(inputs["bo"], C6),
        "boc_col": col(inputs["boc"], C6),
        "bf2_col": col(inputs["bf2"], C6),
        "bf1_col": col(inputs["bf1"], F24),
        "bq_colb": colb(inputs["bq"], C6),
        "bqc_colb": colb(inputs["bqc"], C6),
        "bv_row": row(inputs["bv"]),
        "bvc_row": row(inputs["bvc"]),
        "g1_row": row(inputs["g1"]), "b1_row": row(inputs["b1"]),
        "g2_row": row(inputs["g2"]), "b2_row": row(inputs["b2"]),
        "g3_row": row(inputs["g3"]), "b3_row": row(inputs["b3"]),
    }
    _NC_CACHE["wb"] = wb
    in_maps = [
        _prep_core_inputs(core // 2, core % 2, inputs["cur_input"],
                          inputs["prevLayerOutput"], inputs["classVector"],
                          inputs["rand_idx"], wb)
        for core in range(8)
    ]
    res = run_bass_kernel_spmd(nc, in_maps, core_ids=list(range(8)))
    out = np.empty((B, S, D), np.float32)
    for core in range(8):
        b, half = core // 2, core % 2
        out[b, half * S2 : (half + 1) * S2] = res.results[core]["out"].T
    return out


if __name__ == "__main__":
    _build_nc()
    print("build ok")
